# revision 1
# baseline (speedup 1.0000x reference)
"""LSTM decoder w/ Luong attention + input feeding, Trainium2 Bass kernel.

T=64 steps, B=64, D=512, S=512, 2-layer LSTM, dot attention, input feed.
Sharding: data-parallel over batch, 8 cores x 8 batches, zero collectives.
All weight matmuls stream the weight as the moving operand (bf16 / f32r ->
1 cycle/row). Attention score matrix memT stays resident in SBUF in f32
(bitcast f32r for the PE); the ctx-orientation copy of memory_bank streams
from DRAM in bf16 each step (hidden under PE time).
"""

import os
import sys

sys.path.insert(0, "/opt/trn_rl_repo")

import numpy as np
import ml_dtypes

T_FULL, B_FULL, D, S, V = 64, 64, 512, 512, 32000
NC = 8
BL = B_FULL // NC  # 8 batches per core
G = 4 * D  # 2048
NK_D = D // 128  # 4
T_STEPS = int(os.environ.get("KERNEL_T", T_FULL))

BF16 = ml_dtypes.bfloat16


def _build(T):
    import concourse.bass as bass
    import concourse.bacc as bacc
    import concourse.tile as tile
    from concourse import mybir

    nc = bacc.Bacc(None, target_bir_lowering=False)
    f32 = mybir.dt.float32
    f32r = mybir.dt.float32r
    bf16 = mybir.dt.bfloat16
    AF = mybir.ActivationFunctionType

    embT_d = nc.dram_tensor("embT", [128, NK_D, T_FULL, BL], bf16, kind="ExternalInput")
    wih0_d = nc.dram_tensor("wih0T", [128, 2 * NK_D, G], bf16, kind="ExternalInput")
    whh0_d = nc.dram_tensor("whh0T", [128, NK_D, G], bf16, kind="ExternalInput")
    wih1_d = nc.dram_tensor("wih1T", [128, NK_D, G], bf16, kind="ExternalInput")
    whh1_d = nc.dram_tensor("whh1T", [128, NK_D, G], bf16, kind="ExternalInput")
    wout_d = nc.dram_tensor("woutT", [128, 2 * NK_D, D], bf16, kind="ExternalInput")
    bias_d = nc.dram_tensor("bias01", [1, 2 * G], bf16, kind="ExternalInput")
    memT_d = nc.dram_tensor("memT", [128, NK_D, BL, S], f32r, kind="ExternalInput")
    memc_d = nc.dram_tensor("memc", [128, NK_D, BL, D], bf16, kind="ExternalInput")
    mask_d = nc.dram_tensor("mask", [128, 2, S], bf16, kind="ExternalInput")
    eye128_d = nc.dram_tensor("eye128", [128, 128], f32, kind="ExternalInput")
    zr_d = nc.dram_tensor("zr", [128, NK_D, 2, BL], f32r, kind="ExternalInput")
    dec_d = nc.dram_tensor("dec_outs", [T_FULL, BL, D], f32, kind="ExternalOutput")
    att_d = nc.dram_tensor("attns", [T_FULL, BL, S], f32, kind="ExternalOutput")

    with tile.TileContext(nc) as tc:
        with (
            tc.tile_pool(name="res", bufs=1) as res,
            tc.tile_pool(name="state", bufs=1) as state,
            tc.tile_pool(name="work", bufs=1) as work,
            tc.tile_pool(name="memcp", bufs=3) as memcp,
            tc.tile_pool(name="pg", bufs=1, space="PSUM") as pg,
            tc.tile_pool(name="pg2", bufs=2, space="PSUM") as pg2,
            tc.tile_pool(name="pt", bufs=2, space="PSUM") as pt,
        ):
            wih0 = res.tile([128, 2 * NK_D, G], bf16)
            nc.sync.dma_start(out=wih0, in_=wih0_d.ap())
            whh0 = res.tile([128, NK_D, G], bf16)
            nc.sync.dma_start(out=whh0, in_=whh0_d.ap())
            wih1 = res.tile([128, NK_D, G], bf16)
            nc.sync.dma_start(out=wih1, in_=wih1_d.ap())
            whh1 = res.tile([128, NK_D, G], bf16)
            nc.sync.dma_start(out=whh1, in_=whh1_d.ap())
            wout = res.tile([128, 2 * NK_D, D], bf16)
            nc.sync.dma_start(out=wout, in_=wout_d.ap())
            bias01 = res.tile([1, 2 * G], bf16)
            nc.sync.dma_start(out=bias01, in_=bias_d.ap())
            memT = res.tile([128, NK_D, BL, S], f32r)
            nc.sync.dma_start(out=memT, in_=memT_d.ap())
            mask = res.tile([128, 2, S], bf16)
            nc.sync.dma_start(out=mask, in_=mask_d.ap())
            eye128 = res.tile([128, 128], f32)
            nc.sync.dma_start(out=eye128, in_=eye128_d.ap())
            ones = res.tile([1, BL], bf16)
            nc.vector.memset(ones, 1.0)

            c0 = state.tile([BL, D], f32)
            c1 = state.tile([BL, D], f32)
            h0T = state.tile([128, NK_D, BL], bf16)
            h1T = state.tile([128, NK_D, BL], bf16)
            h1Tr2 = state.tile([128, NK_D, 2, BL], f32r)
            feedT = state.tile([128, NK_D, BL], bf16)
            for t_ in (c0, c1, h0T, h1T, feedT):
                nc.vector.memset(t_, 0.0)
            nc.sync.dma_start(out=h1Tr2, in_=zr_d.ap())

            IFO = 3 * D

            def transpose_8xD(src_sb, outs, dup_out=None):
                """src [8,512] f32 SBUF -> each out tile [128,NK_D,8] (cast).
                dup_out: [128,NK_D,2,BL] tile receiving doubled columns."""
                tp = pt.tile([128, NK_D, BL], f32, tag="tp")
                for k in range(NK_D):
                    nc.tensor.transpose(
                        tp[:, k, :], src_sb[:, k * 128 : (k + 1) * 128],
                        eye128[0:BL, 0:BL],
                    )
                for o in outs:
                    nc.vector.tensor_copy(o, tp)
                if dup_out is not None:
                    tv = tp[:, :, :]
                    dup = bass.AP(tensor=tv.tensor, offset=tv.offset,
                                  ap=[tv.ap[0], tv.ap[1], [0, 2], tv.ap[2]])
                    nc.vector.tensor_copy(dup_out, dup)

            def lstm_cell(gps, cprev, houts, dup_out=None):
                sig = work.tile([BL, IFO], f32, tag="sig")
                nc.scalar.activation(sig, gps[:, 0:IFO], AF.Sigmoid)
                tg = work.tile([BL, D], f32, tag="tg")
                nc.scalar.activation(tg, gps[:, IFO:G], AF.Tanh)
                fc = work.tile([BL, D], f32, tag="tc")
                nc.vector.tensor_mul(fc, sig[:, D : 2 * D], cprev)
                ig = work.tile([BL, D], f32, tag="h")
                nc.vector.tensor_mul(ig, sig[:, 0:D], tg)
                nc.vector.tensor_add(cprev, fc, ig)
                tc_ = work.tile([BL, D], f32, tag="tc")
                nc.scalar.activation(tc_, cprev, AF.Tanh)
                h = work.tile([BL, D], f32, tag="h")
                nc.vector.tensor_mul(h, sig[:, 2 * D : IFO], tc_)
                transpose_8xD(h, houts, dup_out=dup_out)

            for t in range(T):
                # ===== layer-0 gates: [emb;feed;1] @ [Wih0.T;b0] + h0@Whh0.T
                g0 = pg.tile([BL, G], f32, tag="gates")
                et = memcp.tile([128, NK_D, BL], bf16, tag="et")
                nc.sync.dma_start(out=et, in_=embT_d.ap()[:, :, t, :])
                for n in range(4):
                    nsl = slice(n * 512, (n + 1) * 512)
                    for k in range(NK_D):
                        nc.tensor.matmul(g0[:, nsl], et[:, k, :],
                                         wih0[:, k, nsl], start=(k == 0), stop=False)
                    for k in range(NK_D):
                        nc.tensor.matmul(g0[:, nsl], feedT[:, k, :],
                                         wih0[:, NK_D + k, nsl], start=False, stop=False)
                    for k in range(NK_D):
                        nc.tensor.matmul(g0[:, nsl], h0T[:, k, :],
                                         whh0[:, k, nsl], start=False, stop=False)
                    nc.tensor.matmul(g0[:, nsl], ones, bias01[:, nsl],
                                     start=False, stop=True)
                lstm_cell(g0, c0, [h0T])

                # ===== layer-1 gates
                g1 = pg.tile([BL, G], f32, tag="gates")
                for n in range(4):
                    nsl = slice(n * 512, (n + 1) * 512)
                    for k in range(NK_D):
                        nc.tensor.matmul(g1[:, nsl], h0T[:, k, :],
                                         wih1[:, k, nsl], start=(k == 0), stop=False)
                    for k in range(NK_D):
                        nc.tensor.matmul(g1[:, nsl], h1T[:, k, :],
                                         whh1[:, k, nsl], start=False, stop=False)
                    nc.tensor.matmul(g1[:, nsl], ones,
                                     bias01[:, G + n * 512 : G + (n + 1) * 512],
                                     start=False, stop=True)
                lstm_cell(g1, c1, [h1T], dup_out=h1Tr2)

                # ===== attention scores (f32r). Rotated dup lhsT puts
                # batch b's row at partition 0; spread out to partition 32j.
                psc = work.tile([128, 2, S], f32, tag="p")
                for b in range(BL):
                    u, j = b // 4, b % 4
                    ob = pg2.tile([BL, S], f32, tag="sc8")
                    for k in range(NK_D):
                        nc.tensor.matmul(
                            ob, h1Tr2[:, k, :, :].rearrange("p a b -> p (a b)")[
                                :, b : b + BL],
                            memT[:, k, b, :],
                            start=(k == 0), stop=(k == NK_D - 1))
                    if b % 2 == 0:
                        nc.vector.tensor_copy(psc[32 * j : 32 * j + 1, u, :],
                                              ob[0:1, :])
                    else:
                        nc.scalar.copy(psc[32 * j : 32 * j + 1, u, :], ob[0:1, :])
                nc.vector.tensor_add(psc, psc, mask)
                nmx = work.tile([128, 2], f32, tag="nmx")
                nc.vector.tensor_reduce(nmx, psc, axis=mybir.AxisListType.X,
                                        op=mybir.AluOpType.max, negate=True)
                ssum = work.tile([128, 2], f32, tag="ssum")
                for u in range(2):
                    nc.scalar.activation(psc[:, u, :], psc[:, u, :], AF.Exp,
                                         bias=nmx[:, u : u + 1], scale=1.0,
                                         accum_out=ssum[:, u : u + 1])
                rs = work.tile([128, 2], f32, tag="rs")
                nc.vector.reciprocal(rs, ssum)
                for u in range(2):
                    nc.vector.tensor_scalar_mul(psc[:, u, :], in0=psc[:, u, :],
                                                scalar1=rs[:, u : u + 1])
                    nc.sync.dma_start(
                        out=att_d.ap()[t, 4 * u : 4 * u + 4, :],
                        in_=psc[0:97:32, u, :])
                # transpose spread p, gather+dup to pT2 [128,NK_D,2*BL] bf16
                pT2 = work.tile([128, NK_D, 2, BL], bf16, tag="pT2")
                for k in range(NK_D):
                    tk = pt.tile([128, 2, 128], f32, tag="tp")
                    for u in range(2):
                        nc.tensor.transpose(
                            tk[:, u, :], psc[:, u, 128 * k : 128 * (k + 1)],
                            eye128)
                    tv = tk[:, :, :]
                    gat = bass.AP(tensor=tv.tensor, offset=tv.offset,
                                  ap=[tv.ap[0], [0, 2], [128, 2], [32, 4]])
                    nc.vector.tensor_copy(pT2[:, k], gat)

                # ===== context: per-batch mem slab streamed from DRAM (bf16)
                cxs = work.tile([128, 2, D], f32, tag="cxs")
                for b in range(BL):
                    u, j = b // 4, b % 4
                    mc = memcp.tile([128, NK_D, D], bf16, tag="mc")
                    nc.sync.dma_start(out=mc, in_=memc_d.ap()[:, :, b, :])
                    cb = pg2.tile([BL, D], f32, tag="sc8")
                    for k in range(NK_D):
                        nc.tensor.matmul(
                            cb, pT2[:, k, :, :].rearrange("p a b -> p (a b)")[
                                :, b : b + BL],
                            mc[:, k, :],
                            start=(k == 0), stop=(k == NK_D - 1))
                    if b % 2 == 0:
                        nc.vector.tensor_copy(cxs[32 * j : 32 * j + 1, u, :],
                                              cb[0:1, :])
                    else:
                        nc.scalar.copy(cxs[32 * j : 32 * j + 1, u, :], cb[0:1, :])
                cxT = work.tile([128, NK_D, 2, 128], bf16, tag="xT")
                for k in range(NK_D):
                    tk = pt.tile([128, 2, 128], f32, tag="tp")
                    for u in range(2):
                        nc.tensor.transpose(
                            tk[:, u, :], cxs[:, u, 128 * k : 128 * (k + 1)],
                            eye128)
                    nc.vector.tensor_copy(cxT[:, k], tk)

                # ===== output projection + tanh
                # lhsT cols (u,j) at free offset 32j of half u -> M=8 in b order
                ah = pt.tile([BL, D], f32, tag="tp")
                for k in range(NK_D):
                    cv = cxT[:, k, :, :]
                    lv = bass.AP(tensor=cv.tensor, offset=cv.offset,
                                 ap=[cv.ap[0], [128, 2], [32, 4]])
                    nc.tensor.matmul(ah[:, :], lv, wout[:, k, :],
                                     start=(k == 0), stop=False)
                for k in range(NK_D):
                    nc.tensor.matmul(ah[:, :], h1T[:, k, :], wout[:, NK_D + k, :],
                                     start=False, stop=(k == NK_D - 1))
                af = work.tile([BL, D], f32, tag="h")
                nc.scalar.activation(af, ah, AF.Tanh)
                nc.sync.dma_start(out=dec_d.ap()[t], in_=af)
                transpose_8xD(af, [feedT])
    nc.compile()
    return nc


def kernel(tokens, memory_bank, memory_lengths, emb_table,
           Wih0, Whh0, bih0, bhh0, Wih1, Whh1, bih1, bhh1, Wout):
    import concourse.tile_utils as tile_utils
    from concourse.bass_utils import run_bass_kernel_spmd

    tile_utils.max_sbuf_usage = 206 * 1024

    tokens = np.asarray(tokens)
    memory_bank = np.asarray(memory_bank, dtype=np.float32)
    memory_lengths = np.asarray(memory_lengths)
    f32 = np.float32

    # gate reorder [i,f,g,o] -> [i,f,o,g]
    perm = np.concatenate([np.arange(0, 2 * D), np.arange(3 * D, 4 * D),
                           np.arange(2 * D, 3 * D)])
    Wih0p, Whh0p = np.asarray(Wih0, f32)[perm], np.asarray(Whh0, f32)[perm]
    Wih1p, Whh1p = np.asarray(Wih1, f32)[perm], np.asarray(Whh1, f32)[perm]
    b0 = (np.asarray(bih0, f32) + np.asarray(bhh0, f32))[perm]
    b1 = (np.asarray(bih1, f32) + np.asarray(bhh1, f32))[perm]
    bias01 = np.concatenate([b0, b1])[None, :].astype(BF16)

    def wT(w, nk):
        return np.ascontiguousarray(
            np.asarray(w, f32).T.reshape(nk, 128, w.shape[0]).transpose(1, 0, 2)
        ).astype(BF16)

    wih0T = wT(Wih0p, 2 * NK_D)
    whh0T, wih1T, whh1T = wT(Whh0p, NK_D), wT(Wih1p, NK_D), wT(Whh1p, NK_D)
    woutT = wT(np.asarray(Wout, f32), 2 * NK_D)
    emb = np.asarray(emb_table, f32)[tokens.astype(np.int64)]  # [T,B,D]

    nc = _build(T_STEPS)

    in_maps = []
    for c in range(NC):
        sl = slice(c * BL, (c + 1) * BL)
        e = emb[:, sl, :]
        embT = np.ascontiguousarray(
            e.reshape(T_FULL, BL, NK_D, 128).transpose(3, 2, 0, 1)).astype(BF16)
        m = memory_bank[:, sl, :]  # [S,8,D]
        memT = np.ascontiguousarray(
            m.reshape(S, BL, NK_D, 128).transpose(3, 2, 1, 0)).astype(f32)
        memc = np.ascontiguousarray(
            m.reshape(NK_D, 128, BL, D).transpose(1, 0, 2, 3)).astype(BF16)
        lens = memory_lengths[sl].astype(np.int64)
        mrow = np.where(np.arange(S)[None, :] < lens[:, None], 0.0,
                        -1e9).astype(f32)  # [8,S]
        mask = np.full((128, 2, S), -1e9, dtype=BF16)
        for b in range(BL):
            mask[32 * (b % 4), b // 4, :] = mrow[b].astype(BF16)
        in_maps.append(dict(
            embT=embT, wih0T=wih0T, whh0T=whh0T, wih1T=wih1T, whh1T=whh1T,
            woutT=woutT, bias01=bias01, memT=memT, memc=memc, mask=mask,
            eye128=np.eye(128, dtype=f32),
            zr=np.zeros((128, NK_D, 2, BL), dtype=f32)))

    res = run_bass_kernel_spmd(
        nc, in_maps, core_ids=list(range(NC)),
        trace=bool(int(os.environ.get("KERNEL_TRACE", "0"))))
    dec = np.concatenate([r["dec_outs"] for r in res.results], axis=1)
    att = np.concatenate([r["attns"] for r in res.results], axis=1)
    globals()["_last_results"] = res
    return dec, att

